# revision 46
# baseline (speedup 1.0000x reference)
"""Topic-aware multi-head attention on 8 Trainium2 cores.

Sharding: batch(4) x head-half(2) -> 8 cores. Each core computes one batch's
attention for 8 of 16 heads and partial output projections over its local
512 context dims; host sums the partials per batch and adds bo.

Schedule (v7; ~210us vs 223us baseline):
  - Chunked input DMAs + c-outer q-projection: the first matmul fires after
    one 128-row chunk of wq/xq lands; a dummy-matmul warm-up block climbs the
    PE p-state ramp during the DMA preamble.
  - Gate trick: host stacks [-G; +G] weight columns so one sigmoid yields
    both p and (1-p) on partitions 0-15; the per-head q-scale broadcast is a
    single K=16 matmul per (head, half).
  - Attention runs as 8 windows paced by Scalar's exp stream. Each window
    opens with most of the previous head's ctx matmuls, interleaves the rest
    behind the scores groups, and runs the previous head's softmax-denominator
    chain (stage1 sums copy / stage2 pack-recip-broadcast / stage3 multiply)
    at fixed kM slots so the in-order Vector queue never stalls and the single
    ctx PSUM buffer is free before the next window needs it.
  - ktproj (hM-major weights) and vproj are emitted as 2-matmul micro-steps
    popped between scores groups per a static per-window plan, bridging the
    exp-paced PSUM waits.
  - exp outputs pair up in [128,2048] tiles so mask multiplies run half as
    many DVE ops; masks use fp16 SBUF operands throughout.
  - Normalized ctx lands in four per-pair tiles (avoids false dependencies
    in the out-projection); heads are processed [0..5,7,6] so the last head's
    norm multiply writes its ctx tile directly (even partition base).
  - Output projection is two DRAM partials summed on host: a merged
    chunk-(0,1,2) partial computed at the tail while the last norm chain
    drains, and the chunk-3 partial right after; copies alternate
    Scalar/Vector and PSUM use alternates two pools.
  - PSUM->SBUF copies are spread across Scalar and Vector so neither paces
    the exp stream; the recip broadcast runs on GpSimd (partition_broadcast,
    proxy ucode library loaded once).
"""
import functools
import numpy as np
from contextlib import ExitStack

import concourse.bass as bass
import concourse.tile as tile
from concourse import bacc, library_config, mybir
from concourse.bass_utils import run_bass_kernel_spmd

F16 = mybir.dt.float16
F32 = mybir.dt.float32
AF = mybir.ActivationFunctionType
ALU = mybir.AluOpType

H, D, DT, DH, B, L = 16, 1024, 100, 64, 4, 1024
NKC = 8   # din chunks (1024/128)
NQ = 2    # 512-wide halves of L


def build_nc():
    nc = bacc.Bacc("TRN2", target_bir_lowering=False)

    def par(name, shape, dt=F16, out=False):
        return nc.declare_dram_parameter(name, list(shape), dt, isOutput=out)

    xq = par("xq", (128, 8192)); xk = par("xk", (128, 8192)); xv = par("xv", (128, 8192))
    top = par("top", (128, 1024))
    mk = par("mk", (128, 8192))
    wq = par("wq", (128, 4096))
    wkc = par("wkc", (128, 8192))      # hM-major: [:, hM*1024 + c*128 + m]
    wv = par("wv", (128, 4096))
    wtv = par("wtv", (128, 512))
    wo = par("wo", (128, 4096))
    gt = par("gt", (128, 272))         # 17 chunks x 16 cols of [-G; +G]
    selS = par("selS", (16, 1024))     # per-head q-scale selector
    btwc = par("btwc", (16, 1), F32)   # [-btw_eff; +btw_eff]
    out = par("out", (128, 16384), F16, out=True)  # 2 partials

    with tile.TileContext(nc) as tc, ExitStack() as ctx:
        cst = ctx.enter_context(tc.tile_pool(name="cst", bufs=1))
        qr = ctx.enter_context(tc.tile_pool(name="qr", bufs=2))
        ep = ctx.enter_context(tc.tile_pool(name="ep", bufs=2))
        op = ctx.enter_context(tc.tile_pool(name="op", bufs=6))
        ctp = ctx.enter_context(tc.tile_pool(name="ctp", bufs=2))
        ps = ctx.enter_context(tc.tile_pool(name="ps", bufs=2, space="PSUM"))
        sm = ctx.enter_context(tc.tile_pool(name="sm", bufs=2, space="PSUM"))
        cxp = ctx.enter_context(tc.tile_pool(name="cxp", bufs=1, space="PSUM"))

        mm = nc.tensor.matmul
        # one gpsimd library that covers tensor_tensor AND partition_broadcast
        nc.gpsimd.load_library(library_config.proxy)

        # ---- all input DMAs, issued in arrival-priority order ----
        wq_c, xq_c = [], []
        for c in range(2):
            wt = cst.tile([128, 512], F16, tag=f"wq{c}", name=f"wq{c}")
            nc.sync.dma_start(out=wt, in_=wq[:, c * 512:(c + 1) * 512])
            wq_c.append(wt)
            xt = cst.tile([128, 1024], F16, tag=f"xq{c}", name=f"xq{c}")
            nc.sync.dma_start(out=xt, in_=xq[:, c * 1024:(c + 1) * 1024])
            xq_c.append(xt)
        gt_t = cst.tile([128, 272], F16, tag="gt")
        nc.sync.dma_start(out=gt_t, in_=gt[:, :])
        selS_t = cst.tile([16, 1024], F16, tag="selS")
        nc.sync.dma_start(out=selS_t, in_=selS[:, :])
        btw_t = cst.tile([16, 1], F32, tag="btw")
        nc.sync.dma_start(out=btw_t, in_=btwc[:, :])
        for c in range(2, NKC):
            wt = cst.tile([128, 512], F16, tag=f"wq{c}", name=f"wq{c}")
            nc.sync.dma_start(out=wt, in_=wq[:, c * 512:(c + 1) * 512])
            wq_c.append(wt)
            xt = cst.tile([128, 1024], F16, tag=f"xq{c}", name=f"xq{c}")
            nc.sync.dma_start(out=xt, in_=xq[:, c * 1024:(c + 1) * 1024])
            xq_c.append(xt)
        top_t = cst.tile([128, 1024], F16, tag="top")
        nc.sync.dma_start(out=top_t, in_=top[:, :])
        wtv_t = cst.tile([128, 512], F16, tag="wtv")
        nc.sync.dma_start(out=wtv_t, in_=wtv[:, :])
        xk_c = []
        for c in range(NKC):
            xt = cst.tile([128, 1024], F16, tag=f"xk{c}")
            nc.sync.dma_start(out=xt, in_=xk[:, c * 1024:(c + 1) * 1024])
            xk_c.append(xt)
        wkc_c = []
        for hM in range(2):
            wt = cst.tile([128, 1024], F16, tag=f"wkc{hM}", name=f"wkc{hM}")
            nc.sync.dma_start(out=wt, in_=wkc[:, hM * 1024:(hM + 1) * 1024])
            wkc_c.append(wt)
        wv_t = cst.tile([128, 4096], F16, tag="wv")
        nc.sync.dma_start(out=wv_t, in_=wv[:, :])
        for hM in range(2, 8):
            wt = cst.tile([128, 1024], F16, tag=f"wkc{hM}", name=f"wkc{hM}")
            nc.sync.dma_start(out=wt, in_=wkc[:, hM * 1024:(hM + 1) * 1024])
            wkc_c.append(wt)
        wo_t = cst.tile([128, 4096], F16, tag="wo")
        nc.sync.dma_start(out=wo_t, in_=wo[:, :])
        # xv chunks overlay the xq chunk slots (free after gate-q/qproj),
        # mask chunks overlay the wq chunk slots (wq is done once qproj ends)
        xv_c = []
        for c in range(NKC):
            xt = cst.tile([128, 1024], F16, tag=f"xq{c}", name=f"xv{c}")
            nc.sync.dma_start(out=xt, in_=xv[:, c * 1024:(c + 1) * 1024])
            xv_c.append(xt)
        mk2_c = []
        for j in range(4):
            mt = cst.tile([128, 2048], F16, tag=f"mk{j}", name=f"mk{j}")
            nc.sync.dma_start(out=mt, in_=mk[:, j * 2048:(j + 1) * 2048])
            mk2_c.append(mt)

        # ---- persistent SBUF ----
        kstc = [cst.tile([128, 1024], F16, tag=f"kst{hh}", name=f"kst{hh}")
                for hh in range(8)]   # [k_h; tk_h] stacked, per head
        qst_t = cst.tile([128, 8192], F16, tag="qst")   # [q_h; tq_h] stacked
        v_t = cst.tile([128, 4160], F16, tag="v")       # (kM, h, 64 v + 1 ones)
        vv = v_t.rearrange("p (k h x) -> p k h x", k=8, h=8)
        ctxc = [cst.tile([128, 1024], F16, tag=f"ctx{m}", name=f"ctx{m}")
                for m in range(4)]
        st_t = cst.tile([16, 1024], F16, tag="st")      # rows 0-7: 1-p, 8-15: p
        packed_t = cst.tile([128, 64], F16, tag="packed")
        rec_t = cst.tile([128, 64], F16, tag="rec")
        nc.vector.memset(vv[:, :, :, 64:65], 1.0)

        gate_p = cxp.tile([16, 1024], F32, tag="cx", name="gate_p")

        # ---- PE warm-up: dummy matmuls on zeroed SBUF so the tensor engine
        # climbs its p-state ramp before the first real projection arrives
        warm_t = cst.tile([128, 512], F16, tag="warm")
        nc.vector.memset(warm_t, 0.0)
        wp_p = ps.tile([128, 512], F32, tag="ps", name="warm_p")
        for i in range(9):
            mm(wp_p[:, :], warm_t[:, 0:128], warm_t[:, :],
               start=(i == 0), stop=(i == 8))

        # ---- phase 1: q projection, c-outer, with gate-q interleaved ----
        for half in range(2):
            pps = [ps.tile([128, 1024], F32, tag="ps", name=f"qp{half}{i}")
                   for i in range(2)]
            for c in range(NKC):
                for mi in range(2):
                    m = half * 2 + mi
                    for qh in range(NQ):
                        mm(pps[mi][:, qh * 512: qh * 512 + 512],
                           wq_c[c][:, m * 128:(m + 1) * 128],
                           xq_c[c][:, qh * 512: qh * 512 + 512],
                           start=(c == 0), stop=(c == NKC - 1))
                if half == 0:
                    for qh in range(NQ):
                        mm(gate_p[:, qh * 512: qh * 512 + 512],
                           gt_t[:, c * 16:(c + 1) * 16],
                           xq_c[c][:, qh * 512: qh * 512 + 512],
                           start=(c == 0), stop=False)
            for mi in range(2):
                m = half * 2 + mi
                qt = qr.tile([128, 1024], F16, tag="qr", name=f"qt{m}")
                nc.scalar.copy(qt[:, :], pps[mi][:, :])
                nc.sync.dma_start(out=qst_t[0:64, (2 * m) * 1024:(2 * m + 1) * 1024],
                                  in_=qt[0:64, :])
                nc.sync.dma_start(out=qst_t[0:64, (2 * m + 1) * 1024:(2 * m + 2) * 1024],
                                  in_=qt[64:128, :])

        # ---- phase 2: topic-q proj; gate xk/top parts; sigmoid ----
        for m in range(4):
            pp2 = ps.tile([128, 1024], F32, tag="ps", name=f"tq{m}")
            for qh in range(NQ):
                mm(pp2[:, qh * 512: qh * 512 + 512], wtv_t[:, m * 128:(m + 1) * 128],
                   top_t[:, qh * 512: qh * 512 + 512], start=True, stop=True)
            # interleave gate xk chunks between tq tiles (2 chunks per m)
            for c in (2 * m, 2 * m + 1):
                for qh in range(NQ):
                    mm(gate_p[:, qh * 512: qh * 512 + 512],
                       gt_t[:, (8 + c) * 16:(9 + c) * 16],
                       xk_c[c][:, qh * 512: qh * 512 + 512],
                       start=False, stop=False)
            qt2 = qr.tile([128, 1024], F16, tag="qr", name=f"qt2{m}")
            nc.scalar.copy(qt2[:, :], pp2[:, :])
            nc.sync.dma_start(out=qst_t[64:128, (2 * m) * 1024:(2 * m + 1) * 1024],
                              in_=qt2[0:64, :])
            nc.sync.dma_start(out=qst_t[64:128, (2 * m + 1) * 1024:(2 * m + 2) * 1024],
                              in_=qt2[64:128, :])
        for qh in range(NQ):
            mm(gate_p[:, qh * 512: qh * 512 + 512], gt_t[:, 16 * 16: 17 * 16],
               top_t[:, qh * 512: qh * 512 + 512], start=False, stop=True)
        nc.scalar.activation(st_t[:, :], gate_p[:, :], AF.Sigmoid, bias=btw_t[:, :])

        # ---- helpers ----
        def ktproj_half(hM, qh):
            pp = sm.tile([128, 512], F32, tag="sm", name=f"kt{hM}{qh}")
            for c in range(NKC):
                mm(pp[:, :],
                   wkc_c[hM][:, c * 128:(c + 1) * 128],
                   xk_c[c][:, qh * 512: qh * 512 + 512],
                   start=(c == 0), stop=(c == NKC - 1))
            dst = kstc[hM][:, qh * 512: qh * 512 + 512]
            if qh == 0:
                nc.scalar.copy(dst, pp[:, :])
            else:
                nc.vector.tensor_copy(dst, pp[:, :])

        def sel_head(h):
            bbp = ps.tile([128, 1024], F32, tag="ps", name=f"sel{h}")
            for qh in range(NQ):
                mm(bbp[:, qh * 512: qh * 512 + 512],
                   selS_t[:, h * 128:(h + 1) * 128],
                   st_t[:, qh * 512: qh * 512 + 512], start=True, stop=True)
            nc.vector.tensor_mul(qst_t[:, h * 1024:(h + 1) * 1024],
                                 qst_t[:, h * 1024:(h + 1) * 1024], bbp[:, :])

        def vproj(lM):
            vp = sm.tile([128, 512], F32, tag="sm", name=f"v{lM}")
            for c in range(NKC):
                mm(vp[:, :],
                   xv_c[c][:, lM * 128:(lM + 1) * 128],
                   wv_t[:, c * 512:(c + 1) * 512], start=(c == 0), stop=(c == NKC - 1))
            src_r = vp.rearrange("p (h x) -> p h x", h=8)
            if lM % 2 == 0:
                nc.scalar.copy(vv[:, lM, :, 0:64], src_r)
            else:
                nc.vector.tensor_copy(vv[:, lM, :, 0:64], src_r)

        COPY_ENG = [nc.scalar, nc.vector]

        def out_lM(slot, chunks, lM, ci, use_ps=False):
            # one [128,1024] partial-output tile: PSUM-accumulates the given
            # ctx chunks, then copies out on alternating engines
            out_t = op.tile([128, 1024], F16, tag="o", name=f"ot{slot}{lM}")
            for qh in range(NQ):
                if use_ps and qh == 0:
                    o_p = ps.tile([128, 512], F32, tag="ps", name=f"o{slot}{lM}{qh}")
                else:
                    o_p = sm.tile([128, 512], F32, tag="sm", name=f"o{slot}{lM}{qh}")
                for i, c in enumerate(chunks):
                    mm(o_p[:, :],
                       ctxc[c][:, lM * 128:(lM + 1) * 128],
                       wo_t[:, c * 1024 + qh * 512: c * 1024 + qh * 512 + 512],
                       start=(i == 0), stop=(i == len(chunks) - 1))
                if (ci + qh) % 2 == 0:
                    nc.vector.tensor_copy(out_t[:, qh * 512: qh * 512 + 512], o_p[:, :])
                else:
                    nc.scalar.copy(out_t[:, qh * 512: qh * 512 + 512], o_p[:, :])
            nc.sync.dma_start(
                out=out[:, slot * 8192 + lM * 1024: slot * 8192 + (lM + 1) * 1024],
                in_=out_t)

        # normalization, split into 3 stages so the DVE queue never stalls
        # waiting on the pack DMA / gpsimd broadcast mid-mask-stream.
        # s1 stages the unnormalized ctx and sums row to SBUF so the single
        # ctx PSUM buffer frees immediately (the next head's ctx block would
        # otherwise wait the whole pack/recip/broadcast chain).
        def norm_s1(h, ctx_p, on_scalar=False):
            # one 65-row copy stages ctx AND the sums row (free-dim bound,
            # so 65 rows cost the same as 64); pack-DMA reads the staged row
            cu = ctp.tile([65, 1024], F16, tag="cu", name=f"cu{h}", bufs=2)
            if on_scalar:
                nc.scalar.copy(cu[:, :], ctx_p[0:65, :])
            else:
                nc.vector.tensor_copy(cu[:, :], ctx_p[0:65, :])
            nc.sync.dma_start(out=packed_t[:, h * 8:(h + 1) * 8],
                              in_=cu[64:65, :])
            return cu

        def norm_s2(h):
            rrh = ctp.tile([1, 1024], F16, tag="rrh", name=f"rrh{h}", bufs=1)
            with nc.allow_low_precision("softmax denominators"):
                nc.vector.reciprocal(rec_t[:, h * 8:(h + 1) * 8],
                                     packed_t[:, h * 8:(h + 1) * 8])
            nc.sync.dma_start(out=rrh[:, :], in_=rec_t[:, h * 8:(h + 1) * 8])
            bc = ctp.tile([64, 1024], F16, tag="bc", name=f"bc{h}")
            nc.gpsimd.partition_broadcast(bc[:, :], rrh[:, :])
            return bc

        def norm_s3(h, cu, bc):
            hm, hr = h // 2, (h % 2) * 64
            if hr == 0:
                # even head: same partition base, multiply straight into ctxc
                nc.vector.tensor_mul(ctxc[hm][0:64, :], cu[0:64, :], bc[:, :])
            else:
                ctmp = ctp.tile([64, 1024], F16, tag="ctmp", name=f"cn{h}")
                nc.vector.tensor_mul(ctmp[:, :], cu[0:64, :], bc[:, :])
                nc.sync.dma_start(out=ctxc[hm][hr:hr + 64, :], in_=ctmp[:, :])

        # ---- phase 3: sel + first kt tiles interleaved ----
        sel_head(0)
        ktproj_half(0, 0)
        sel_head(1)
        ktproj_half(0, 1)
        sel_head(2)
        ktproj_half(1, 0)
        sel_head(3)
        ktproj_half(1, 1)
        for h in (4, 5, 6, 7):
            sel_head(h)

        # ---- phase 4: attention with statically scheduled filler work ----
        # Each window h: (a) all ctx(h-1) mms back-to-back so its norm chain
        # starts ~7us earlier (cxp bufs=1 then never blocks ctx(h) next
        # window), (b) scores(h) groups paced by Scalar's exp stream with
        # filler items popped between groups, per a static per-window plan.
        def kt_micro(hM, qh):
            # 4 micro-steps of 2 accumulating matmuls; copy on the last
            st = {"c": 0}

            def step():
                if st["c"] == 0:
                    st["pp"] = sm.tile([128, 512], F32, tag="sm",
                                       name=f"kt{hM}{qh}")
                pp = st["pp"]
                for c in (st["c"], st["c"] + 1):
                    mm(pp[:, :],
                       wkc_c[hM][:, c * 128:(c + 1) * 128],
                       xk_c[c][:, qh * 512: qh * 512 + 512],
                       start=(c == 0), stop=(c == NKC - 1))
                st["c"] += 2
                if st["c"] == 8:
                    # keep Scalar's exp stream uninterrupted: copies on DVE
                    nc.vector.tensor_copy(
                        kstc[hM][:, qh * 512: qh * 512 + 512], pp[:, :])
            return [step] * 4

        def v_micro(lM):
            st = {"c": 0}

            def step():
                if st["c"] == 0:
                    st["vp"] = sm.tile([128, 512], F32, tag="sm",
                                       name=f"v{lM}")
                vp = st["vp"]
                for c in (st["c"], st["c"] + 1):
                    mm(vp[:, :],
                       xv_c[c][:, lM * 128:(lM + 1) * 128],
                       wv_t[:, c * 512:(c + 1) * 512],
                       start=(c == 0), stop=(c == NKC - 1))
                st["c"] += 2
                if st["c"] == 8:
                    nc.vector.tensor_copy(
                        vv[:, lM, :, 0:64],
                        vp.rearrange("p (h x) -> p h x", h=8))
            return [step] * 4

        def outB_micro(slot, chunks, lM, ci, use_ps=False):
            # one micro-step per query-half: accumulate chunks + copy out
            st = {"qh": 0}
            out_t_ref = {}

            def step():
                qh = st["qh"]
                if qh == 0:
                    out_t_ref["t"] = op.tile([128, 1024], F16, tag="o",
                                             name=f"ot{slot}{lM}")
                out_t = out_t_ref["t"]
                if use_ps and qh == 0:
                    o_p = ps.tile([128, 512], F32, tag="ps",
                                  name=f"o{slot}{lM}{qh}")
                else:
                    o_p = sm.tile([128, 512], F32, tag="sm",
                                  name=f"o{slot}{lM}{qh}")
                for i, c in enumerate(chunks):
                    mm(o_p[:, :],
                       ctxc[c][:, lM * 128:(lM + 1) * 128],
                       wo_t[:, c * 1024 + qh * 512: c * 1024 + qh * 512 + 512],
                       start=(i == 0), stop=(i == len(chunks) - 1))
                if (ci + qh) % 2 == 0:
                    nc.vector.tensor_copy(out_t[:, qh * 512: qh * 512 + 512],
                                          o_p[:, :])
                else:
                    nc.scalar.copy(out_t[:, qh * 512: qh * 512 + 512],
                                   o_p[:, :])
                st["qh"] += 1
                if st["qh"] == 2:
                    nc.sync.dma_start(
                        out=out[:, slot * 8192 + lM * 1024:
                                slot * 8192 + (lM + 1) * 1024],
                        in_=out_t)
            return [step, step]

        # process heads so the LAST one is even (direct ctx_t norm write)
        PORD = [0, 1, 2, 3, 4, 5, 7, 6]
        STATIC = {
            0: kt_micro(2, 0) + sum([v_micro(lM) for lM in range(8)], []),
            1: kt_micro(2, 1) + kt_micro(3, 0),
            2: kt_micro(3, 1) + kt_micro(4, 0),
            3: kt_micro(4, 1) + kt_micro(5, 0),
            4: kt_micro(5, 1) + kt_micro(7, 0),
            5: kt_micro(7, 1) + kt_micro(6, 0),
            6: kt_micro(6, 1),
            7: [],
        }
        fillers = []
        fidx = [0]

        def pop_filler(n):
            while n > 0 and fidx[0] < len(fillers):
                fillers[fidx[0]]()
                fidx[0] += 1
                n -= 1

        def ctx_pair(ph, pctx, pems, kM):
            emt, off = pems[kM]
            for qh in range(NQ):
                mm(pctx[0:65, qh * 512: qh * 512 + 512],
                   v_t[:, (kM * 8 + ph) * 65: (kM * 8 + ph) * 65 + 65],
                   emt[:, off + qh * 512: off + qh * 512 + 512],
                   start=(kM == 0), stop=(kM == 7))

        prev = None  # (h, ctx_p, ems)
        for i in range(8):
            h = PORD[i]
            ctx_p = cxp.tile([128, 1024], F32, tag="cx", name=f"ctx{h}")
            if prev is not None:
                # half of prev head's ctx as the window-opening block
                for kM in range(4):
                    ctx_pair(prev[0], prev[1], prev[2], kM)
            fillers.extend(STATIC[i])
            budget = len(STATIC[i])
            bc_prev = None
            cu_prev = None
            ems = {}
            e2 = None
            for kM in range(8):
                sp = ps.tile([128, 1024], F32, tag="ps", name=f"sp{h}{kM}")
                for qh in range(NQ):
                    mm(sp[:, qh * 512: qh * 512 + 512],
                       kstc[h][:, kM * 128:(kM + 1) * 128],
                       qst_t[:, h * 1024 + qh * 512: h * 1024 + qh * 512 + 512],
                       start=True, stop=True)
                if kM % 2 == 0:
                    e2 = ep.tile([128, 2048], F16, tag="e", name="e2", bufs=2)
                nc.scalar.activation(e2[:, (kM % 2) * 1024:(kM % 2) * 1024 + 1024],
                                     sp[:, :], AF.Exp)
                if kM % 2 == 1:
                    em2 = ep.tile([128, 2048], F16, tag="em", name="em2", bufs=5)
                    nc.vector.tensor_mul(em2[:, :], e2[:, :], mk2_c[kM // 2][:, :])
                    ems[kM - 1] = (em2, 0)
                    ems[kM] = (em2, 1024)
                if prev is not None:
                    # second half of prev ctx bridges the exp-paced sp waits
                    # one pair per slot; its stop lands at slot 3 and the cu
                    # copy (k4) frees the single cxp buffer mid-window
                    if kM < 4:
                        ctx_pair(prev[0], prev[1], prev[2], kM + 4)
                    elif kM == 4:
                        cu_prev = norm_s1(prev[0], prev[1])
                    elif kM == 5:
                        bc_prev = norm_s2(prev[0])
                    elif kM == 7:
                        norm_s3(prev[0], cu_prev, bc_prev)
                if i == 0:
                    take = (budget * (kM + 1)) // 8 - (budget * kM) // 8
                elif kM >= 4:
                    # slots 0-3 carry the interleaved ctx pairs; fillers go
                    # where only scores run against the exp cadence
                    take = (budget * (kM - 3)) // 4 - (budget * (kM - 4)) // 4
                else:
                    take = 0
                pop_filler(take)
            prev = (h, ctx_p, ems)

        # ---- tail: ctx(last) dense, norm chain interleaved with partial 0,
        # then out3 once the last head's ctx lands in ctx_t ----
        ph, pctx, pems = prev
        for kM in range(8):
            ctx_pair(ph, pctx, pems, kM)
        cu_l = norm_s1(ph, pctx, on_scalar=True)
        for lM in range(8):
            for s in outB_micro(0, (0, 1, 2), lM, lM, use_ps=(lM % 2 == 0)):
                s()
        bc_l = norm_s2(ph)
        norm_s3(ph, cu_l, bc_l)
        for lM in range(8):
            for s in outB_micro(1, (3,), lM, lM, use_ps=(lM % 2 == 0)):
                s()

    nc.compile()
    return nc


@functools.lru_cache(maxsize=1)
def _nc_cached():
    return build_nc()


def _chunk128(a):
    # [R, C] -> [128, (R/128)*C] grouping row-chunks of 128 into the free dim
    r, c = a.shape
    return np.ascontiguousarray(
        a.reshape(r // 128, 128, c).transpose(1, 0, 2).reshape(128, (r // 128) * c))


def prepare_in_maps(inputs):
    inp = {k: np.asarray(v) for k, v in inputs.items()}
    query, key, value = inp["query"], inp["key"], inp["value"]
    mask, topic = inp["mask"], inp["topic_vec"]
    Wq, bq, Wk, bk, Wv, bv = inp["Wq"], inp["bq"], inp["Wk"], inp["bk"], inp["Wv"], inp["bv"]
    Wtk, btk, Wtv, btv = inp["Wtk"], inp["btk"], inp["Wtv"], inp["btv"]
    Wtw, btw, Wo, bo = inp["Wtw"], inp["btw"], inp["Wo"], inp["bo"]

    f16 = np.float16
    # selS: per-head q-scale selector, K=16 ([1-p; p] stacked in st_t)
    selS = np.zeros((16, 8, 128), np.float32)
    for h in range(8):
        selS[h, h, :64] = 1.0        # content half scaled by (1-p)
        selS[8 + h, h, 64:] = 1.0    # topic half scaled by p
    selS = selS.reshape(16, 1024)

    Gq = Wtw[:, :D] @ Wq
    Gk = Wtw[:, D:2 * D] @ Wtk
    Gt = Wtw[:, 2 * D:] @ Wtv
    btw_eff = btw + Wtw[:, :D] @ bq + Wtw[:, D:2 * D] @ btk + Wtw[:, 2 * D:] @ btv

    in_maps = []
    for core in range(8):
        b = core // 2
        hh = (core % 2)
        hs = slice(hh * 8, hh * 8 + 8)
        ds_ = slice(hh * 512, hh * 512 + 512)

        topT = np.zeros((128, L), np.float32)
        topT[:DT] = topic[b].T
        wtvT = np.zeros((128, 512), np.float32)
        wtvT[:DT] = Wtv[ds_].T / 8
        gT = np.concatenate(
            [Gq[hs].T, Gk[hs].T, np.pad(Gt[hs].T, ((0, 28), (0, 0)))], 0)  # [2176, 8]
        gT16 = np.concatenate([-gT, gT], 1)  # [2176, 16]
        btw16 = np.concatenate([-btw_eff[hs], btw_eff[hs]]).reshape(16, 1)

        # stacked per-head [content-k(64); topic-k(64)] weights
        Wk_l, Wtk_l = Wk[ds_], Wtk[ds_]
        wkcomb = np.zeros((1024, D), np.float32)
        for h in range(8):
            wkcomb[h * 128: h * 128 + 64] = Wk_l[h * 64:(h + 1) * 64]
            wkcomb[h * 128 + 64: h * 128 + 128] = Wtk_l[h * 64:(h + 1) * 64]
        # hM-major layout: [:, hM*1024 + c*128 + m]
        wkcT = wkcomb.T.reshape(8, 128, 8, 128).transpose(1, 2, 0, 3).reshape(128, 8192)

        m = {
            "xq": _chunk128(query[b].T).astype(f16),
            "xk": _chunk128(key[b].T).astype(f16),
            "xv": _chunk128(value[b].T).astype(f16),
            "top": topT.astype(f16),
            "mk": _chunk128(
                np.where(mask[b].T, np.float32(0), np.float32(1))).astype(f16),
            "wq": _chunk128(Wq[ds_].T / 8).astype(f16),
            "wkc": np.ascontiguousarray(wkcT).astype(f16),
            "wv": _chunk128(Wv[ds_].T).astype(f16),
            "wtv": wtvT.astype(f16),
            "wo": _chunk128(Wo[:, ds_].T).astype(f16),
            "gt": _chunk128(gT16).astype(f16),
            "selS": selS.astype(f16),
            "btwc": btw16.astype(np.float32),
        }
        in_maps.append(m)
    return in_maps, bo


def gather_out(results, bo):
    out_full = np.zeros((B, L, D), np.float32)
    for core in range(8):
        b = core // 2
        o = results[core]["out"].astype(np.float32)  # [128, 2*8192] fp16 partials
        o = o.reshape(128, 2, 8, 1024).sum(1)        # sum the 2 partials
        o = o.transpose(1, 0, 2).reshape(1024, 1024)
        out_full[b] += o
    out_full += bo.astype(np.float32)
    return out_full


def kernel(**inputs):
    in_maps, bo = prepare_in_maps(inputs)
    nc = _nc_cached()
    res = run_bass_kernel_spmd(nc, in_maps, list(range(8)))
    return gather_out(res.results, bo)


# revision 47
# speedup vs baseline: 1.1836x; 1.1836x over previous
"""Topic-aware multi-head attention on 8 Trainium2 cores.

Sharding: batch(4) x head-half(2) -> 8 cores. Each core computes one batch's
attention for 8 of 16 heads and partial output projections over its local
512 context dims; host sums the partials per batch and adds bo.

Schedule (v7; ~210us vs 223us baseline):
  - Chunked input DMAs + c-outer q-projection: the first matmul fires after
    one 128-row chunk of wq/xq lands; a dummy-matmul warm-up block climbs the
    PE p-state ramp during the DMA preamble.
  - Gate trick: host stacks [-G; +G] weight columns so one sigmoid yields
    both p and (1-p) on partitions 0-15; the per-head q-scale broadcast is a
    single K=16 matmul per (head, half).
  - Attention runs as 8 windows paced by Scalar's exp stream. Each window
    opens with most of the previous head's ctx matmuls, interleaves the rest
    behind the scores groups, and runs the previous head's softmax-denominator
    chain (stage1 sums copy / stage2 pack-recip-broadcast / stage3 multiply)
    at fixed kM slots so the in-order Vector queue never stalls and the single
    ctx PSUM buffer is free before the next window needs it.
  - ktproj (hM-major weights) and vproj are emitted as 2-matmul micro-steps
    popped between scores groups per a static per-window plan, bridging the
    exp-paced PSUM waits.
  - exp outputs pair up in [128,2048] tiles so mask multiplies run half as
    many DVE ops; masks use fp16 SBUF operands throughout.
  - Normalized ctx lands in four per-pair tiles (avoids false dependencies
    in the out-projection); heads are processed [0..5,7,6] so the last head's
    norm multiply writes its ctx tile directly (even partition base).
  - Output projection is two DRAM partials summed on host: a merged
    chunk-(0,1,2) partial computed at the tail while the last norm chain
    drains, and the chunk-3 partial right after; copies alternate
    Scalar/Vector and PSUM use alternates two pools.
  - PSUM->SBUF copies are spread across Scalar and Vector so neither paces
    the exp stream; the recip broadcast runs on GpSimd (partition_broadcast,
    proxy ucode library loaded once).
"""
import functools
import numpy as np
from contextlib import ExitStack

import concourse.bass as bass
import concourse.tile as tile
from concourse import bacc, library_config, mybir
from concourse.bass_utils import run_bass_kernel_spmd

F16 = mybir.dt.float16
F32 = mybir.dt.float32
AF = mybir.ActivationFunctionType
ALU = mybir.AluOpType

H, D, DT, DH, B, L = 16, 1024, 100, 64, 4, 1024
NKC = 8   # din chunks (1024/128)
NQ = 2    # 512-wide halves of L


def build_nc():
    nc = bacc.Bacc("TRN2", target_bir_lowering=False)

    def par(name, shape, dt=F16, out=False):
        return nc.declare_dram_parameter(name, list(shape), dt, isOutput=out)

    xq = par("xq", (128, 8192)); xk = par("xk", (128, 8192)); xv = par("xv", (128, 8192))
    top = par("top", (128, 1024))
    mk = par("mk", (128, 8192))
    wq = par("wq", (128, 4096))
    wkc = par("wkc", (128, 8192))      # hM-major: [:, hM*1024 + c*128 + m]
    wv = par("wv", (128, 4096))
    wtv = par("wtv", (128, 512))
    wo = par("wo", (128, 4096))
    gt = par("gt", (128, 272))         # 17 chunks x 16 cols of [-G; +G]
    selS = par("selS", (16, 1024))     # per-head q-scale selector
    btwc = par("btwc", (16, 1), F32)   # [-btw_eff; +btw_eff]
    out = par("out", (128, 16384), F16, out=True)  # 2 partials

    with tile.TileContext(nc) as tc, ExitStack() as ctx:
        cst = ctx.enter_context(tc.tile_pool(name="cst", bufs=1))
        qr = ctx.enter_context(tc.tile_pool(name="qr", bufs=2))
        ep = ctx.enter_context(tc.tile_pool(name="ep", bufs=2))
        op = ctx.enter_context(tc.tile_pool(name="op", bufs=6))
        ctp = ctx.enter_context(tc.tile_pool(name="ctp", bufs=2))
        ps = ctx.enter_context(tc.tile_pool(name="ps", bufs=2, space="PSUM"))
        sm = ctx.enter_context(tc.tile_pool(name="sm", bufs=2, space="PSUM"))
        cxp = ctx.enter_context(tc.tile_pool(name="cxp", bufs=1, space="PSUM"))

        mm = nc.tensor.matmul
        # one gpsimd library that covers tensor_tensor AND partition_broadcast
        nc.gpsimd.load_library(library_config.proxy)

        # ---- all input DMAs, issued in arrival-priority order ----
        wq_c, xq_c = [], []
        for c in range(2):
            wt = cst.tile([128, 512], F16, tag=f"wq{c}", name=f"wq{c}")
            nc.sync.dma_start(out=wt, in_=wq[:, c * 512:(c + 1) * 512])
            wq_c.append(wt)
            xt = cst.tile([128, 1024], F16, tag=f"xq{c}", name=f"xq{c}")
            nc.sync.dma_start(out=xt, in_=xq[:, c * 1024:(c + 1) * 1024])
            xq_c.append(xt)
        gt_t = cst.tile([128, 272], F16, tag="gt")
        nc.sync.dma_start(out=gt_t, in_=gt[:, :])
        selS_t = cst.tile([16, 1024], F16, tag="selS")
        nc.sync.dma_start(out=selS_t, in_=selS[:, :])
        btw_t = cst.tile([16, 1], F32, tag="btw")
        nc.sync.dma_start(out=btw_t, in_=btwc[:, :])
        for c in range(2, NKC):
            wt = cst.tile([128, 512], F16, tag=f"wq{c}", name=f"wq{c}")
            nc.sync.dma_start(out=wt, in_=wq[:, c * 512:(c + 1) * 512])
            wq_c.append(wt)
            xt = cst.tile([128, 1024], F16, tag=f"xq{c}", name=f"xq{c}")
            nc.sync.dma_start(out=xt, in_=xq[:, c * 1024:(c + 1) * 1024])
            xq_c.append(xt)
        top_t = cst.tile([128, 1024], F16, tag="top")
        nc.sync.dma_start(out=top_t, in_=top[:, :])
        wtv_t = cst.tile([128, 512], F16, tag="wtv")
        nc.sync.dma_start(out=wtv_t, in_=wtv[:, :])
        xk_c = []
        for c in range(NKC):
            xt = cst.tile([128, 1024], F16, tag=f"xk{c}")
            nc.sync.dma_start(out=xt, in_=xk[:, c * 1024:(c + 1) * 1024])
            xk_c.append(xt)
        wkc_c = []
        for hM in range(2):
            wt = cst.tile([128, 1024], F16, tag=f"wkc{hM}", name=f"wkc{hM}")
            nc.sync.dma_start(out=wt, in_=wkc[:, hM * 1024:(hM + 1) * 1024])
            wkc_c.append(wt)
        wv_t = cst.tile([128, 4096], F16, tag="wv")
        nc.sync.dma_start(out=wv_t, in_=wv[:, :])
        for hM in range(2, 8):
            wt = cst.tile([128, 1024], F16, tag=f"wkc{hM}", name=f"wkc{hM}")
            nc.sync.dma_start(out=wt, in_=wkc[:, hM * 1024:(hM + 1) * 1024])
            wkc_c.append(wt)
        wo_t = cst.tile([128, 4096], F16, tag="wo")
        nc.sync.dma_start(out=wo_t, in_=wo[:, :])
        # xv chunks overlay the xq chunk slots (free after gate-q/qproj),
        # mask chunks overlay the wq chunk slots (wq is done once qproj ends)
        xv_c = []
        for c in range(NKC):
            xt = cst.tile([128, 1024], F16, tag=f"xq{c}", name=f"xv{c}")
            nc.sync.dma_start(out=xt, in_=xv[:, c * 1024:(c + 1) * 1024])
            xv_c.append(xt)
        mk2_c = []
        for j in range(4):
            mt = cst.tile([128, 2048], F16, tag=f"mk{j}", name=f"mk{j}")
            nc.sync.dma_start(out=mt, in_=mk[:, j * 2048:(j + 1) * 2048])
            mk2_c.append(mt)

        # ---- persistent SBUF ----
        kstc = [cst.tile([128, 1024], F16, tag=f"kst{hh}", name=f"kst{hh}")
                for hh in range(8)]   # [k_h; tk_h] stacked, per head
        qst_t = cst.tile([128, 8192], F16, tag="qst")   # [q_h; tq_h] stacked
        v_t = cst.tile([128, 4160], F16, tag="v")       # (kM, h, 64 v + 1 ones)
        vv = v_t.rearrange("p (k h x) -> p k h x", k=8, h=8)
        ctxc = [cst.tile([128, 1024], F16, tag=f"ctx{m}", name=f"ctx{m}")
                for m in range(4)]
        st_t = cst.tile([16, 1024], F16, tag="st")      # rows 0-7: 1-p, 8-15: p
        packed_t = cst.tile([128, 64], F16, tag="packed")
        rec_t = cst.tile([128, 64], F16, tag="rec")
        nc.vector.memset(vv[:, :, :, 64:65], 1.0)

        gate_p = cxp.tile([16, 1024], F32, tag="cx", name="gate_p")

        # ---- PE warm-up: dummy matmuls on zeroed SBUF so the tensor engine
        # climbs its p-state ramp before the first real projection arrives
        warm_t = cst.tile([128, 512], F16, tag="warm")
        nc.vector.memset(warm_t, 0.0)
        wp_p = ps.tile([128, 512], F32, tag="ps", name="warm_p")
        for i in range(14):
            mm(wp_p[:, :], warm_t[:, 0:128], warm_t[:, :],
               start=(i == 0), stop=(i == 13))

        # ---- phase 1: q projection, c-outer, with gate-q interleaved ----
        for half in range(2):
            pps = [ps.tile([128, 1024], F32, tag="ps", name=f"qp{half}{i}")
                   for i in range(2)]
            for c in range(NKC):
                for mi in range(2):
                    m = half * 2 + mi
                    for qh in range(NQ):
                        mm(pps[mi][:, qh * 512: qh * 512 + 512],
                           wq_c[c][:, m * 128:(m + 1) * 128],
                           xq_c[c][:, qh * 512: qh * 512 + 512],
                           start=(c == 0), stop=(c == NKC - 1))
                if half == 0:
                    for qh in range(NQ):
                        mm(gate_p[:, qh * 512: qh * 512 + 512],
                           gt_t[:, c * 16:(c + 1) * 16],
                           xq_c[c][:, qh * 512: qh * 512 + 512],
                           start=(c == 0), stop=False)
            for mi in range(2):
                m = half * 2 + mi
                qt = qr.tile([128, 1024], F16, tag="qr", name=f"qt{m}")
                nc.scalar.copy(qt[:, :], pps[mi][:, :])
                nc.sync.dma_start(out=qst_t[0:64, (2 * m) * 1024:(2 * m + 1) * 1024],
                                  in_=qt[0:64, :])
                nc.sync.dma_start(out=qst_t[0:64, (2 * m + 1) * 1024:(2 * m + 2) * 1024],
                                  in_=qt[64:128, :])

        # ---- phase 2: topic-q proj; gate xk/top parts; sigmoid ----
        for m in range(4):
            pp2 = ps.tile([128, 1024], F32, tag="ps", name=f"tq{m}")
            for qh in range(NQ):
                mm(pp2[:, qh * 512: qh * 512 + 512], wtv_t[:, m * 128:(m + 1) * 128],
                   top_t[:, qh * 512: qh * 512 + 512], start=True, stop=True)
            # interleave gate xk chunks between tq tiles (2 chunks per m)
            for c in (2 * m, 2 * m + 1):
                for qh in range(NQ):
                    mm(gate_p[:, qh * 512: qh * 512 + 512],
                       gt_t[:, (8 + c) * 16:(9 + c) * 16],
                       xk_c[c][:, qh * 512: qh * 512 + 512],
                       start=False, stop=False)
            qt2 = qr.tile([128, 1024], F16, tag="qr", name=f"qt2{m}")
            nc.scalar.copy(qt2[:, :], pp2[:, :])
            nc.sync.dma_start(out=qst_t[64:128, (2 * m) * 1024:(2 * m + 1) * 1024],
                              in_=qt2[0:64, :])
            nc.sync.dma_start(out=qst_t[64:128, (2 * m + 1) * 1024:(2 * m + 2) * 1024],
                              in_=qt2[64:128, :])
        for qh in range(NQ):
            mm(gate_p[:, qh * 512: qh * 512 + 512], gt_t[:, 16 * 16: 17 * 16],
               top_t[:, qh * 512: qh * 512 + 512], start=False, stop=True)
        nc.scalar.activation(st_t[:, :], gate_p[:, :], AF.Sigmoid, bias=btw_t[:, :])

        # ---- helpers ----
        def ktproj_half(hM, qh):
            pp = sm.tile([128, 512], F32, tag="sm", name=f"kt{hM}{qh}")
            for c in range(NKC):
                mm(pp[:, :],
                   wkc_c[hM][:, c * 128:(c + 1) * 128],
                   xk_c[c][:, qh * 512: qh * 512 + 512],
                   start=(c == 0), stop=(c == NKC - 1))
            dst = kstc[hM][:, qh * 512: qh * 512 + 512]
            if qh == 0:
                nc.scalar.copy(dst, pp[:, :])
            else:
                nc.vector.tensor_copy(dst, pp[:, :])

        def sel_head(h):
            bbp = ps.tile([128, 1024], F32, tag="ps", name=f"sel{h}")
            for qh in range(NQ):
                mm(bbp[:, qh * 512: qh * 512 + 512],
                   selS_t[:, h * 128:(h + 1) * 128],
                   st_t[:, qh * 512: qh * 512 + 512], start=True, stop=True)
            nc.vector.tensor_mul(qst_t[:, h * 1024:(h + 1) * 1024],
                                 qst_t[:, h * 1024:(h + 1) * 1024], bbp[:, :])

        def vproj(lM):
            vp = sm.tile([128, 512], F32, tag="sm", name=f"v{lM}")
            for c in range(NKC):
                mm(vp[:, :],
                   xv_c[c][:, lM * 128:(lM + 1) * 128],
                   wv_t[:, c * 512:(c + 1) * 512], start=(c == 0), stop=(c == NKC - 1))
            src_r = vp.rearrange("p (h x) -> p h x", h=8)
            if lM % 2 == 0:
                nc.scalar.copy(vv[:, lM, :, 0:64], src_r)
            else:
                nc.vector.tensor_copy(vv[:, lM, :, 0:64], src_r)

        COPY_ENG = [nc.scalar, nc.vector]

        def out_lM(slot, chunks, lM, ci, use_ps=False):
            # one [128,1024] partial-output tile: PSUM-accumulates the given
            # ctx chunks, then copies out on alternating engines
            out_t = op.tile([128, 1024], F16, tag="o", name=f"ot{slot}{lM}")
            for qh in range(NQ):
                if use_ps and qh == 0:
                    o_p = ps.tile([128, 512], F32, tag="ps", name=f"o{slot}{lM}{qh}")
                else:
                    o_p = sm.tile([128, 512], F32, tag="sm", name=f"o{slot}{lM}{qh}")
                for i, c in enumerate(chunks):
                    mm(o_p[:, :],
                       ctxc[c][:, lM * 128:(lM + 1) * 128],
                       wo_t[:, c * 1024 + qh * 512: c * 1024 + qh * 512 + 512],
                       start=(i == 0), stop=(i == len(chunks) - 1))
                if (ci + qh) % 2 == 0:
                    nc.vector.tensor_copy(out_t[:, qh * 512: qh * 512 + 512], o_p[:, :])
                else:
                    nc.scalar.copy(out_t[:, qh * 512: qh * 512 + 512], o_p[:, :])
            nc.sync.dma_start(
                out=out[:, slot * 8192 + lM * 1024: slot * 8192 + (lM + 1) * 1024],
                in_=out_t)

        # normalization, split into 3 stages so the DVE queue never stalls
        # waiting on the pack DMA / gpsimd broadcast mid-mask-stream.
        # s1 stages the unnormalized ctx and sums row to SBUF so the single
        # ctx PSUM buffer frees immediately (the next head's ctx block would
        # otherwise wait the whole pack/recip/broadcast chain).
        def norm_s1(h, ctx_p, on_scalar=False):
            # one 65-row copy stages ctx AND the sums row (free-dim bound,
            # so 65 rows cost the same as 64); pack-DMA reads the staged row
            cu = ctp.tile([65, 1024], F16, tag="cu", name=f"cu{h}", bufs=2)
            if on_scalar:
                nc.scalar.copy(cu[:, :], ctx_p[0:65, :])
            else:
                nc.vector.tensor_copy(cu[:, :], ctx_p[0:65, :])
            nc.sync.dma_start(out=packed_t[:, h * 8:(h + 1) * 8],
                              in_=cu[64:65, :])
            return cu

        def norm_s2(h):
            rrh = ctp.tile([1, 1024], F16, tag="rrh", name=f"rrh{h}", bufs=1)
            with nc.allow_low_precision("softmax denominators"):
                nc.vector.reciprocal(rec_t[:, h * 8:(h + 1) * 8],
                                     packed_t[:, h * 8:(h + 1) * 8])
            nc.sync.dma_start(out=rrh[:, :], in_=rec_t[:, h * 8:(h + 1) * 8])
            bc = ctp.tile([64, 1024], F16, tag="bc", name=f"bc{h}")
            nc.gpsimd.partition_broadcast(bc[:, :], rrh[:, :])
            return bc

        def norm_s3(h, cu, bc):
            hm, hr = h // 2, (h % 2) * 64
            if hr == 0:
                # even head: same partition base, multiply straight into ctxc
                nc.vector.tensor_mul(ctxc[hm][0:64, :], cu[0:64, :], bc[:, :])
            else:
                ctmp = ctp.tile([64, 1024], F16, tag="ctmp", name=f"cn{h}")
                nc.vector.tensor_mul(ctmp[:, :], cu[0:64, :], bc[:, :])
                nc.sync.dma_start(out=ctxc[hm][hr:hr + 64, :], in_=ctmp[:, :])

        # ---- phase 3: sel + first kt tiles interleaved ----
        sel_head(0)
        ktproj_half(0, 0)
        sel_head(1)
        ktproj_half(0, 1)
        sel_head(2)
        ktproj_half(1, 0)
        sel_head(3)
        ktproj_half(1, 1)
        for h in (4, 5, 6, 7):
            sel_head(h)

        # ---- phase 4: attention with statically scheduled filler work ----
        # Each window h: (a) all ctx(h-1) mms back-to-back so its norm chain
        # starts ~7us earlier (cxp bufs=1 then never blocks ctx(h) next
        # window), (b) scores(h) groups paced by Scalar's exp stream with
        # filler items popped between groups, per a static per-window plan.
        def kt_micro(hM, qh):
            # 4 micro-steps of 2 accumulating matmuls; copy on the last
            st = {"c": 0}

            def step():
                if st["c"] == 0:
                    st["pp"] = sm.tile([128, 512], F32, tag="sm",
                                       name=f"kt{hM}{qh}")
                pp = st["pp"]
                for c in (st["c"], st["c"] + 1):
                    mm(pp[:, :],
                       wkc_c[hM][:, c * 128:(c + 1) * 128],
                       xk_c[c][:, qh * 512: qh * 512 + 512],
                       start=(c == 0), stop=(c == NKC - 1))
                st["c"] += 2
                if st["c"] == 8:
                    # keep Scalar's exp stream uninterrupted: copies on DVE
                    nc.vector.tensor_copy(
                        kstc[hM][:, qh * 512: qh * 512 + 512], pp[:, :])
            return [step] * 4

        def v_micro(lM):
            st = {"c": 0}

            def step():
                if st["c"] == 0:
                    st["vp"] = sm.tile([128, 512], F32, tag="sm",
                                       name=f"v{lM}")
                vp = st["vp"]
                for c in (st["c"], st["c"] + 1):
                    mm(vp[:, :],
                       xv_c[c][:, lM * 128:(lM + 1) * 128],
                       wv_t[:, c * 512:(c + 1) * 512],
                       start=(c == 0), stop=(c == NKC - 1))
                st["c"] += 2
                if st["c"] == 8:
                    nc.vector.tensor_copy(
                        vv[:, lM, :, 0:64],
                        vp.rearrange("p (h x) -> p h x", h=8))
            return [step] * 4

        def outB_micro(slot, chunks, lM, ci, use_ps=False):
            # one micro-step per query-half: accumulate chunks + copy out
            st = {"qh": 0}
            out_t_ref = {}

            def step():
                qh = st["qh"]
                if qh == 0:
                    out_t_ref["t"] = op.tile([128, 1024], F16, tag="o",
                                             name=f"ot{slot}{lM}")
                out_t = out_t_ref["t"]
                if use_ps and qh == 0:
                    o_p = ps.tile([128, 512], F32, tag="ps",
                                  name=f"o{slot}{lM}{qh}")
                else:
                    o_p = sm.tile([128, 512], F32, tag="sm",
                                  name=f"o{slot}{lM}{qh}")
                for i, c in enumerate(chunks):
                    mm(o_p[:, :],
                       ctxc[c][:, lM * 128:(lM + 1) * 128],
                       wo_t[:, c * 1024 + qh * 512: c * 1024 + qh * 512 + 512],
                       start=(i == 0), stop=(i == len(chunks) - 1))
                if (ci + qh) % 2 == 0:
                    nc.vector.tensor_copy(out_t[:, qh * 512: qh * 512 + 512],
                                          o_p[:, :])
                else:
                    nc.scalar.copy(out_t[:, qh * 512: qh * 512 + 512],
                                   o_p[:, :])
                st["qh"] += 1
                if st["qh"] == 2:
                    nc.sync.dma_start(
                        out=out[:, slot * 8192 + lM * 1024:
                                slot * 8192 + (lM + 1) * 1024],
                        in_=out_t)
            return [step, step]

        # process heads so the LAST one is even (direct ctx_t norm write)
        PORD = [0, 1, 2, 3, 4, 5, 7, 6]
        STATIC = {
            0: kt_micro(2, 0) + sum([v_micro(lM) for lM in range(8)], []),
            1: kt_micro(2, 1) + kt_micro(3, 0),
            2: kt_micro(3, 1) + kt_micro(4, 0),
            3: kt_micro(4, 1) + kt_micro(5, 0),
            4: kt_micro(5, 1) + kt_micro(7, 0),
            5: kt_micro(7, 1) + kt_micro(6, 0),
            6: kt_micro(6, 1),
            7: [],
        }
        fillers = []
        fidx = [0]

        def pop_filler(n):
            while n > 0 and fidx[0] < len(fillers):
                fillers[fidx[0]]()
                fidx[0] += 1
                n -= 1

        def ctx_pair(ph, pctx, pems, kM):
            emt, off = pems[kM]
            for qh in range(NQ):
                mm(pctx[0:65, qh * 512: qh * 512 + 512],
                   v_t[:, (kM * 8 + ph) * 65: (kM * 8 + ph) * 65 + 65],
                   emt[:, off + qh * 512: off + qh * 512 + 512],
                   start=(kM == 0), stop=(kM == 7))

        prev = None  # (h, ctx_p, ems)
        for i in range(8):
            h = PORD[i]
            ctx_p = cxp.tile([128, 1024], F32, tag="cx", name=f"ctx{h}")
            if prev is not None:
                # half of prev head's ctx as the window-opening block
                for kM in range(4):
                    ctx_pair(prev[0], prev[1], prev[2], kM)
            fillers.extend(STATIC[i])
            budget = len(STATIC[i])
            bc_prev = None
            cu_prev = None
            ems = {}
            e2 = None
            for kM in range(8):
                sp = ps.tile([128, 1024], F32, tag="ps", name=f"sp{h}{kM}")
                for qh in range(NQ):
                    mm(sp[:, qh * 512: qh * 512 + 512],
                       kstc[h][:, kM * 128:(kM + 1) * 128],
                       qst_t[:, h * 1024 + qh * 512: h * 1024 + qh * 512 + 512],
                       start=True, stop=True)
                if kM % 2 == 0:
                    e2 = ep.tile([128, 2048], F16, tag="e", name="e2", bufs=2)
                nc.scalar.activation(e2[:, (kM % 2) * 1024:(kM % 2) * 1024 + 1024],
                                     sp[:, :], AF.Exp)
                if kM % 2 == 1:
                    em2 = ep.tile([128, 2048], F16, tag="em", name="em2", bufs=5)
                    nc.vector.tensor_mul(em2[:, :], e2[:, :], mk2_c[kM // 2][:, :])
                    ems[kM - 1] = (em2, 0)
                    ems[kM] = (em2, 1024)
                if prev is not None:
                    # second half of prev ctx bridges the exp-paced sp waits
                    # one pair per slot; its stop lands at slot 3 and the cu
                    # copy (k4) frees the single cxp buffer mid-window
                    if kM < 4:
                        ctx_pair(prev[0], prev[1], prev[2], kM + 4)
                    elif kM == 4:
                        cu_prev = norm_s1(prev[0], prev[1])
                    elif kM == 5:
                        bc_prev = norm_s2(prev[0])
                    elif kM == 7:
                        norm_s3(prev[0], cu_prev, bc_prev)
                if i == 0:
                    take = (budget * (kM + 1)) // 8 - (budget * kM) // 8
                elif kM >= 4:
                    # slots 0-3 carry the interleaved ctx pairs; fillers go
                    # where only scores run against the exp cadence
                    take = (budget * (kM - 3)) // 4 - (budget * (kM - 4)) // 4
                else:
                    take = 0
                pop_filler(take)
            prev = (h, ctx_p, ems)

        # ---- tail: ctx(last) dense, norm chain interleaved with partial 0,
        # then out3 once the last head's ctx lands in ctx_t ----
        ph, pctx, pems = prev
        for kM in range(8):
            ctx_pair(ph, pctx, pems, kM)
        cu_l = norm_s1(ph, pctx, on_scalar=True)
        for lM in range(8):
            for s in outB_micro(0, (0, 1, 2), lM, lM, use_ps=(lM % 2 == 0)):
                s()
        bc_l = norm_s2(ph)
        norm_s3(ph, cu_l, bc_l)
        for lM in range(8):
            for s in outB_micro(1, (3,), lM, lM, use_ps=(lM % 2 == 0)):
                s()

    nc.compile()
    return nc


@functools.lru_cache(maxsize=1)
def _nc_cached():
    return build_nc()


def _chunk128(a):
    # [R, C] -> [128, (R/128)*C] grouping row-chunks of 128 into the free dim
    r, c = a.shape
    return np.ascontiguousarray(
        a.reshape(r // 128, 128, c).transpose(1, 0, 2).reshape(128, (r // 128) * c))


def prepare_in_maps(inputs):
    inp = {k: np.asarray(v) for k, v in inputs.items()}
    query, key, value = inp["query"], inp["key"], inp["value"]
    mask, topic = inp["mask"], inp["topic_vec"]
    Wq, bq, Wk, bk, Wv, bv = inp["Wq"], inp["bq"], inp["Wk"], inp["bk"], inp["Wv"], inp["bv"]
    Wtk, btk, Wtv, btv = inp["Wtk"], inp["btk"], inp["Wtv"], inp["btv"]
    Wtw, btw, Wo, bo = inp["Wtw"], inp["btw"], inp["Wo"], inp["bo"]

    f16 = np.float16
    # selS: per-head q-scale selector, K=16 ([1-p; p] stacked in st_t)
    selS = np.zeros((16, 8, 128), np.float32)
    for h in range(8):
        selS[h, h, :64] = 1.0        # content half scaled by (1-p)
        selS[8 + h, h, 64:] = 1.0    # topic half scaled by p
    selS = selS.reshape(16, 1024)

    Gq = Wtw[:, :D] @ Wq
    Gk = Wtw[:, D:2 * D] @ Wtk
    Gt = Wtw[:, 2 * D:] @ Wtv
    btw_eff = btw + Wtw[:, :D] @ bq + Wtw[:, D:2 * D] @ btk + Wtw[:, 2 * D:] @ btv

    in_maps = []
    for core in range(8):
        b = core // 2
        hh = (core % 2)
        hs = slice(hh * 8, hh * 8 + 8)
        ds_ = slice(hh * 512, hh * 512 + 512)

        topT = np.zeros((128, L), np.float32)
        topT[:DT] = topic[b].T
        wtvT = np.zeros((128, 512), np.float32)
        wtvT[:DT] = Wtv[ds_].T / 8
        gT = np.concatenate(
            [Gq[hs].T, Gk[hs].T, np.pad(Gt[hs].T, ((0, 28), (0, 0)))], 0)  # [2176, 8]
        gT16 = np.concatenate([-gT, gT], 1)  # [2176, 16]
        btw16 = np.concatenate([-btw_eff[hs], btw_eff[hs]]).reshape(16, 1)

        # stacked per-head [content-k(64); topic-k(64)] weights
        Wk_l, Wtk_l = Wk[ds_], Wtk[ds_]
        wkcomb = np.zeros((1024, D), np.float32)
        for h in range(8):
            wkcomb[h * 128: h * 128 + 64] = Wk_l[h * 64:(h + 1) * 64]
            wkcomb[h * 128 + 64: h * 128 + 128] = Wtk_l[h * 64:(h + 1) * 64]
        # hM-major layout: [:, hM*1024 + c*128 + m]
        wkcT = wkcomb.T.reshape(8, 128, 8, 128).transpose(1, 2, 0, 3).reshape(128, 8192)

        m = {
            "xq": _chunk128(query[b].T).astype(f16),
            "xk": _chunk128(key[b].T).astype(f16),
            "xv": _chunk128(value[b].T).astype(f16),
            "top": topT.astype(f16),
            "mk": _chunk128(
                np.where(mask[b].T, np.float32(0), np.float32(1))).astype(f16),
            "wq": _chunk128(Wq[ds_].T / 8).astype(f16),
            "wkc": np.ascontiguousarray(wkcT).astype(f16),
            "wv": _chunk128(Wv[ds_].T).astype(f16),
            "wtv": wtvT.astype(f16),
            "wo": _chunk128(Wo[:, ds_].T).astype(f16),
            "gt": _chunk128(gT16).astype(f16),
            "selS": selS.astype(f16),
            "btwc": btw16.astype(np.float32),
        }
        in_maps.append(m)
    return in_maps, bo


def gather_out(results, bo):
    out_full = np.zeros((B, L, D), np.float32)
    for core in range(8):
        b = core // 2
        o = results[core]["out"].astype(np.float32)  # [128, 2*8192] fp16 partials
        o = o.reshape(128, 2, 8, 1024).sum(1)        # sum the 2 partials
        o = o.transpose(1, 0, 2).reshape(1024, 1024)
        out_full[b] += o
    out_full += bo.astype(np.float32)
    return out_full


def kernel(**inputs):
    in_maps, bo = prepare_in_maps(inputs)
    nc = _nc_cached()
    res = run_bass_kernel_spmd(nc, in_maps, list(range(8)))
    return gather_out(res.results, bo)
